# revision 6
# baseline (speedup 1.0000x reference)
"""FDLoss kernel for Trainium2 (Bass/Tile), data-parallel over 8 NeuronCores.

Math (a = target.flatten(), b = source.flatten()):
    fdback = where(a<0 & b<0, b-a, a-b)
    loss   = mean((fdback - a)^2)
Per element:
    fdback - a = (b - 2a) if (a<0 & b<0) else (-b)
    (fdback - a)^2 = (b - 2*a*m)^2,  m = (a<0)&(b<0)
    Let r = relu(-2a)  (= -2a if a<0 else 0, always >= 0)
        w = (b<0) * r  (= -2a if a<0&b<0 else 0)
        t = b + w
    value = t^2
Each core reduces its shard to 128x N_TILES partial sums; host sums in f64.

Sharding: flatten both tensors, split the element dim contiguously across
8 cores (data-parallel per the sharding hint); partial sums come back to the
host which does the final (tiny) reduction instead of an on-device
all-reduce — the output is a scalar, so the gather is 4 KiB total.
"""

import numpy as np

import concourse.bacc as bacc
import concourse.mybir as mybir
from concourse.tile import TileContext
from concourse.bass_utils import run_bass_kernel_spmd

N_CORES = 8
FULL_SHAPE = (64, 256, 56, 56)
TOTAL = 64 * 256 * 56 * 56          # 51,380,224
PER_CORE = TOTAL // N_CORES         # 6,422,528 = 128 * 50,176
P = 128
FD_TOTAL = PER_CORE // P            # 50,176
N_TILES = 14
FD = FD_TOTAL // N_TILES            # 3,584

_F32 = mybir.dt.float32

_cached_nc = None


def _build_bass():
    """Build the single-core SPMD Bass program (same NEFF on all 8 cores)."""
    nc = bacc.Bacc(trn_type="TRN2")

    a_d = nc.dram_tensor("t_in", (PER_CORE,), _F32, kind="ExternalInput")
    b_d = nc.dram_tensor("s_in", (PER_CORE,), _F32, kind="ExternalInput")
    out_d = nc.dram_tensor("partials", (P, N_TILES), _F32, kind="ExternalOutput")

    a_t = a_d.rearrange("(n p m) -> n p m", p=P, m=FD)
    b_t = b_d.rearrange("(n p m) -> n p m", p=P, m=FD)

    with TileContext(nc) as tc:
        with (
            tc.tile_pool(name="a", bufs=3) as a_pool,
            tc.tile_pool(name="b", bufs=3) as b_pool,
            tc.tile_pool(name="w", bufs=3) as w_pool,
            tc.tile_pool(name="s", bufs=3) as s_pool,
            tc.tile_pool(name="acc", bufs=1) as acc_pool,
        ):
            acc = acc_pool.tile([P, N_TILES], _F32)
            for i in range(N_TILES):
                at = a_pool.tile([P, FD], _F32)
                bt = b_pool.tile([P, FD], _F32)
                wt = w_pool.tile([P, FD], _F32)
                st = s_pool.tile([P, FD], _F32)
                nc.sync.dma_start(out=at[:], in_=a_t[i])
                nc.sync.dma_start(out=bt[:], in_=b_t[i])
                # DVE (2x tensor_scalar): s = (b < 0)
                nc.vector.tensor_scalar(
                    out=st[:],
                    in0=bt[:],
                    scalar1=0.0,
                    scalar2=None,
                    op0=mybir.AluOpType.is_lt,
                )
                # ACT: w = relu(-2a)
                nc.scalar.activation(
                    wt[:], at[:], mybir.ActivationFunctionType.Relu, scale=-2.0
                )
                # DVE: w = s * w
                nc.vector.tensor_mul(out=wt[:], in0=st[:], in1=wt[:])
                # DVE: w = b + w   (= t)
                nc.vector.tensor_add(out=wt[:], in0=bt[:], in1=wt[:])
                # ACT: square + accumulate along free dim -> acc[:, i]
                nc.scalar.activation(
                    wt[:],
                    wt[:],
                    mybir.ActivationFunctionType.Square,
                    accum_out=acc[:, i : i + 1],
                )
            nc.sync.dma_start(out=out_d[:], in_=acc[:])

    nc.compile()
    return nc


def _get_nc():
    global _cached_nc
    if _cached_nc is None:
        _cached_nc = _build_bass()
    return _cached_nc


def kernel_impl(source, target, trace=False, **run_kwargs):
    """Returns (loss_scalar_f32, BassKernelResults)."""
    a = np.ascontiguousarray(np.asarray(target, dtype=np.float32).reshape(-1))
    b = np.ascontiguousarray(np.asarray(source, dtype=np.float32).reshape(-1))
    assert a.size == TOTAL and b.size == TOTAL

    in_maps = [
        {
            "t_in": a[i * PER_CORE : (i + 1) * PER_CORE],
            "s_in": b[i * PER_CORE : (i + 1) * PER_CORE],
        }
        for i in range(N_CORES)
    ]

    nc = _get_nc()
    res = run_bass_kernel_spmd(
        nc, in_maps, core_ids=list(range(N_CORES)), trace=trace, **run_kwargs
    )
    total = np.float64(0.0)
    for r in res.results:
        total += r["partials"].astype(np.float64).sum()
    loss = np.float32(total / TOTAL)
    return np.array(loss, dtype=np.float32), res


def kernel(**inputs) -> np.ndarray:
    out, _ = kernel_impl(inputs["source"], inputs["target"])
    return out
